# revision 7
# baseline (speedup 1.0000x reference)
"""Coattention model kernel for 8 Trainium2 NeuronCores.

Data-parallel over batch (B=16 -> 2 samples/core).  Per sample:
  e_corr = w_e @ e, q_corr = w_q @ q           (1x1 convs, bf16 matmuls)
  A[n,m] = e_corr[:,n] . q_corr[:,m]           (PSUM f32)
  E_B = exp(A - rowmax(A)),  rB = 1/rowsum     (softmax over m)
  E_A = exp(A^T - rowmax),   rA = 1/rowsum     (softmax over n, via A^T)
  exemplar_att = (q^T * rB)^T-matmul E_B       (scale folded into qT)
  query_att    = (e^T * rA)^T-matmul E_A
  out[0:512]   = conv3x3(exemplar_att, w_c1)   (9 shifted matmuls, SAME pad)
  out[512:1024]= conv3x3(query_att,    w_c2)

Host pre-transposes/casts all operands so the device does no layout work.
"""

import os
import sys
import types
import numpy as np
import ml_dtypes

import concourse.bass as bass
import concourse.mybir as mybir
import concourse.tile as tile
from concourse import bacc
from concourse.bass_utils import run_bass_kernel_spmd

BF16 = mybir.dt.bfloat16
F32 = mybir.dt.float32
AX = mybir.AxisListType.X
EXP = mybir.ActivationFunctionType.Exp

B, C, HH, WW = 16, 512, 40, 40
N = HH * WW                      # 1600
NCORES = 8
BS = B // NCORES                 # 2 samples per core
CP = 256                         # correlation dim
CC = C // 128                    # 4 channel chunks
PC = CP // 128                   # 2 correlation chunks
NB = (N + 127) // 128            # 13 row blocks (12x128 + 64)
NCH = [(0, 512), (512, 512), (1024, 512), (1536, 64)]   # free-dim chunks of N
YCH = [(0, 12), (12, 12), (24, 12), (36, 4)]            # output row chunks
PW = WW + 2                      # 42
NPAD = PW * PW                   # 1764


def _rows(b):
    return 128 if b < NB - 1 else N - 128 * (NB - 1)


_CACHED = None


def _build_program():
    # COATT_LIMIT: debug stage cutoff ("proj", "apass", "att", "" = full)
    limit = os.environ.get("COATT_LIMIT", "")
    nbs = int(os.environ.get("COATT_BS", str(BS)))
    nc = bacc.Bacc("TRN2", target_bir_lowering=False, debug=False,
                   num_devices=NCORES)

    e_d = nc.dram_tensor("e", [BS, C, N], BF16, kind="ExternalInput")
    q_d = nc.dram_tensor("q", [BS, C, N], BF16, kind="ExternalInput")
    et_d = nc.dram_tensor("et", [BS, N, C], BF16, kind="ExternalInput")
    qt_d = nc.dram_tensor("qt", [BS, N, C], BF16, kind="ExternalInput")
    wet_d = nc.dram_tensor("wet", [C, CP], BF16, kind="ExternalInput")
    wqt_d = nc.dram_tensor("wqt", [C, CP], BF16, kind="ExternalInput")
    wc1_d = nc.dram_tensor("wc1", [C, 9 * C], BF16, kind="ExternalInput")
    wc2_d = nc.dram_tensor("wc2", [C, 9 * C], BF16, kind="ExternalInput")
    out_d = nc.dram_tensor("out", [BS, 2 * C, N], F32, kind="ExternalOutput")

    with tile.TileContext(nc) as tc:
        with (
            tc.tile_pool(name="wproj", bufs=2) as wproj_p,
            tc.tile_pool(name="scratch", bufs=8) as scratch_p,
            tc.tile_pool(name="eqt", bufs=2) as eqt_p,
            tc.tile_pool(name="corr", bufs=2) as corr_p,
            tc.tile_pool(name="big", bufs=2) as big_p,
            tc.tile_pool(name="stats", bufs=16) as stats_p,
            tc.tile_pool(name="ostage", bufs=4) as ostage_p,
            tc.tile_pool(name="mm", bufs=8, space=bass.MemorySpace.PSUM) as mm_p,
        ):
            # --- projection weights, loaded once -------------------------
            wet_t = wproj_p.tile([128, CC * CP], BF16, tag="wproj")
            wqt_t = wproj_p.tile([128, CC * CP], BF16, tag="wproj")
            for cc in range(CC):
                nc.sync.dma_start(wet_t[:, cc * CP:(cc + 1) * CP],
                                  wet_d[cc * 128:(cc + 1) * 128, :])
                nc.sync.dma_start(wqt_t[:, cc * CP:(cc + 1) * CP],
                                  wqt_d[cc * 128:(cc + 1) * 128, :])

            def _dbg_out(s, tiles_bf16):
                """Debug-mode: write bf16 tiles into out so stages stay live."""
                for idx, t in enumerate(tiles_bf16):
                    st = ostage_p.tile([128, 512], F32, tag="ostage")
                    nc.vector.tensor_copy(st[:, :], t[:, :512])
                    nc.sync.dma_start(out_d[s, (idx % 8) * 128:(idx % 8 + 1) * 128,
                                            :512], st[:, :])

            for s in range(nbs):
                # --- input loads ----------------------------------------
                e_t = []
                q_t = []
                for cc in range(CC):
                    t = scratch_p.tile([128, N], BF16, tag="scratch")
                    nc.sync.dma_start(t[:, :], e_d[s, cc * 128:(cc + 1) * 128, :])
                    e_t.append(t)
                for cc in range(CC):
                    t = scratch_p.tile([128, N], BF16, tag="scratch")
                    nc.sync.dma_start(t[:, :], q_d[s, cc * 128:(cc + 1) * 128, :])
                    q_t.append(t)
                et_t = eqt_p.tile([128, NB * C], BF16, tag="eqt")
                qt_t = eqt_p.tile([128, NB * C], BF16, tag="eqt")
                for b in range(NB):
                    r = _rows(b)
                    nc.sync.dma_start(et_t[:r, b * C:(b + 1) * C],
                                      et_d[s, b * 128:b * 128 + r, :])
                    nc.sync.dma_start(qt_t[:r, b * C:(b + 1) * C],
                                      qt_d[s, b * 128:b * 128 + r, :])

                # --- 1x1 projections ------------------------------------
                ecorr_t = corr_p.tile([128, PC * N], BF16, tag="corr")
                qcorr_t = corr_p.tile([128, PC * N], BF16, tag="corr")
                for (w_t, x_t, o_t) in ((wet_t, e_t, ecorr_t),
                                        (wqt_t, q_t, qcorr_t)):
                    for oc in range(PC):
                        for (n0, nw) in NCH:
                            ps = mm_p.tile([128, 512], F32, tag="mm")
                            for cc in range(CC):
                                nc.tensor.matmul(
                                    ps[:, :nw],
                                    w_t[:, cc * CP + oc * 128: cc * CP + (oc + 1) * 128],
                                    x_t[cc][:, n0:n0 + nw],
                                    start=(cc == 0), stop=(cc == CC - 1))
                            nc.scalar.copy(o_t[:, oc * N + n0: oc * N + n0 + nw],
                                           ps[:, :nw])

                # --- A-pass: matmul + streaming softmax -----------------
                def a_pass(lcorr, rcorr, E_t, tgt_t):
                    """E_t = exp(lcorr^T rcorr - rowmax); tgt rows *= 1/rowsum."""
                    for b in range(NB):
                        r = _rows(b)
                        cm = stats_p.tile([128, 4], F32, tag="cm")
                        chunk_ps = []
                        for k, (n0, nw) in enumerate(NCH):
                            ps = mm_p.tile([128, 512], F32, tag="mm")
                            for cc in range(PC):
                                nc.tensor.matmul(
                                    ps[:r, :nw],
                                    lcorr[:, cc * N + b * 128: cc * N + b * 128 + r],
                                    rcorr[:, cc * N + n0: cc * N + n0 + nw],
                                    start=(cc == 0), stop=(cc == PC - 1))
                            nc.vector.reduce_max(cm[:r, k:k + 1], ps[:r, :nw],
                                                 axis=AX)
                            chunk_ps.append(ps)
                        nm = stats_p.tile([128, 1], F32, tag="nm")
                        nc.vector.reduce_max(nm[:r, :], cm[:r, :], axis=AX,
                                             negate=True)
                        sp = stats_p.tile([128, 4], F32, tag="sp")
                        for k, (n0, nw) in enumerate(NCH):
                            nc.scalar.activation(
                                E_t[:r, b * N + n0: b * N + n0 + nw],
                                chunk_ps[k][:r, :nw], EXP,
                                bias=nm[:r, :], accum_out=sp[:r, k:k + 1])
                        rs = stats_p.tile([128, 1], F32, tag="rs")
                        nc.vector.reduce_sum(rs[:r, :], sp[:r, :], axis=AX)
                        rc = stats_p.tile([128, 1], F32, tag="rc")
                        nc.vector.reciprocal(rc[:r, :], rs[:r, :])
                        nc.vector.tensor_scalar_mul(
                            tgt_t[:r, b * C:(b + 1) * C],
                            tgt_t[:r, b * C:(b + 1) * C], rc[:r, :])

                # --- attention matmul into padded images ----------------
                def att(tgtT_t, E_t, pads):
                    for oc in range(CC):
                        pad3 = pads[oc].rearrange("p (a b) -> p a b", a=PW)
                        for (y0, ny) in YCH:
                            nw = ny * WW
                            ps = mm_p.tile([128, 512], F32, tag="mm")
                            for b in range(NB):
                                r = _rows(b)
                                nc.tensor.matmul(
                                    ps[:, :nw],
                                    tgtT_t[:r, b * C + oc * 128: b * C + (oc + 1) * 128],
                                    E_t[:r, b * N + y0 * WW: b * N + y0 * WW + nw],
                                    start=(b == 0), stop=(b == NB - 1))
                            nc.scalar.copy(
                                pad3[:, 1 + y0:1 + y0 + ny, 1:1 + WW],
                                ps[:, :nw].rearrange("p (a b) -> p a b", a=ny))

                # --- 3x3 conv from padded images ------------------------
                def conv(w_t, pads, out_base):
                    pad3s = [p.rearrange("p (a b) -> p a b", a=PW) for p in pads]
                    for oc in range(CC):
                        for (y0, ny) in YCH:
                            nw = ny * WW
                            ps = mm_p.tile([128, 512], F32, tag="mm")
                            idx = 0
                            for ic in range(CC):
                                for d in range(9):
                                    dy, dx = d // 3, d % 3
                                    nc.tensor.matmul(
                                        ps[:, :nw].rearrange("p (a b) -> p a b", a=ny),
                                        w_t[:, ic * 9 * C + d * C + oc * 128:
                                            ic * 9 * C + d * C + (oc + 1) * 128],
                                        pad3s[ic][:, y0 + dy:y0 + dy + ny,
                                                  dx:dx + WW],
                                        start=(idx == 0), stop=(idx == 9 * CC - 1))
                                    idx += 1
                            st = ostage_p.tile([128, 512], F32, tag="ostage")
                            nc.vector.tensor_copy(st[:, :nw], ps[:, :nw])
                            nc.sync.dma_start(
                                out_d[s, out_base + oc * 128: out_base + (oc + 1) * 128,
                                      y0 * WW: y0 * WW + nw],
                                st[:, :nw])

                # E_B / exemplar_att / conv1 then E_A / query_att / conv2.
                if limit == "proj":
                    _dbg_out(s, [ecorr_t, qcorr_t])
                    continue
                EB_t = big_p.tile([128, NB * N], BF16, tag="big")
                a_pass(ecorr_t, qcorr_t, EB_t, qt_t)
                if limit == "apass":
                    _dbg_out(s, [EB_t, qt_t])
                    continue
                pads_e = []
                for oc in range(CC):
                    t = scratch_p.tile([128, NPAD], BF16, tag="scratch")
                    nc.gpsimd.memset(t[:, :], 0.0)
                    pads_e.append(t)
                att(qt_t, EB_t, pads_e)

                EA_t = big_p.tile([128, NB * N], BF16, tag="big")
                a_pass(qcorr_t, ecorr_t, EA_t, et_t)
                pads_q = []
                for oc in range(CC):
                    t = scratch_p.tile([128, NPAD], BF16, tag="scratch")
                    nc.gpsimd.memset(t[:, :], 0.0)
                    pads_q.append(t)
                att(et_t, EA_t, pads_q)

                if limit == "att":
                    _dbg_out(s, pads_e + pads_q)
                    continue
                wc1_t = big_p.tile([128, CC * 9 * C], BF16, tag="big")
                for ic in range(CC):
                    nc.sync.dma_start(wc1_t[:, ic * 9 * C:(ic + 1) * 9 * C],
                                      wc1_d[ic * 128:(ic + 1) * 128, :])
                conv(wc1_t, pads_e, 0)

                wc2_t = big_p.tile([128, CC * 9 * C], BF16, tag="big")
                for ic in range(CC):
                    nc.sync.dma_start(wc2_t[:, ic * 9 * C:(ic + 1) * 9 * C],
                                      wc2_d[ic * 128:(ic + 1) * 128, :])
                conv(wc2_t, pads_q, C)

    nc.compile()
    return nc


def _get_program():
    global _CACHED
    if _CACHED is None:
        _CACHED = _build_program()
    return _CACHED


def _install_ntff_hook():
    """Register the axon NTFF profiling hook if the shim module is absent."""
    if "antenv.axon_hooks" in sys.modules:
        return
    try:
        import antenv
        from trn_agent_boot.trn_boot import _ntff_profile_via_ctypes
    except ImportError:
        return
    mod = types.ModuleType("antenv.axon_hooks")
    _h = [None]
    mod.get_axon_ntff_profile_hook = lambda: _h[0]
    mod.set_axon_ntff_profile_hook = lambda v: _h.__setitem__(0, v)
    sys.modules["antenv.axon_hooks"] = mod
    antenv.axon_hooks = mod
    so = "/opt/axon/libaxon_pjrt.so"
    if os.path.exists(so):
        mod.set_axon_ntff_profile_hook(_ntff_profile_via_ctypes(so))


LAST_RESULTS = None  # BassKernelResults of the most recent run (for test.py)


def prep_in_maps(exemplar, query, w_e, w_q, w_c1, w_c2):
    bf = ml_dtypes.bfloat16
    ex = np.asarray(exemplar, dtype=np.float32).reshape(B, C, N)
    qu = np.asarray(query, dtype=np.float32).reshape(B, C, N)
    e_b = ex.astype(bf)
    q_b = qu.astype(bf)
    et_b = np.ascontiguousarray(e_b.transpose(0, 2, 1))
    qt_b = np.ascontiguousarray(q_b.transpose(0, 2, 1))
    wet = np.ascontiguousarray(
        np.asarray(w_e, dtype=np.float32).reshape(CP, C).T.astype(bf))
    wqt = np.ascontiguousarray(
        np.asarray(w_q, dtype=np.float32).reshape(CP, C).T.astype(bf))
    # [O,I,3,3] -> [I, ky*3+kx, O] -> [I, 9*O]
    wc1 = np.ascontiguousarray(
        np.asarray(w_c1, dtype=np.float32).transpose(1, 2, 3, 0)
        .reshape(C, 9 * C).astype(bf))
    wc2 = np.ascontiguousarray(
        np.asarray(w_c2, dtype=np.float32).transpose(1, 2, 3, 0)
        .reshape(C, 9 * C).astype(bf))

    in_maps = []
    for k in range(NCORES):
        s0 = k * BS
        in_maps.append({
            "e": e_b[s0:s0 + BS], "q": q_b[s0:s0 + BS],
            "et": et_b[s0:s0 + BS], "qt": qt_b[s0:s0 + BS],
            "wet": wet, "wqt": wqt, "wc1": wc1, "wc2": wc2,
        })
    return in_maps


def kernel(exemplar, query, w_e, w_q, w_c1, w_c2):
    in_maps = prep_in_maps(exemplar, query, w_e, w_q, w_c1, w_c2)
    nc = _get_program()
    res = run_bass_kernel_spmd(nc, in_maps, core_ids=list(range(NCORES)),
                               trace=False)
    global LAST_RESULTS
    LAST_RESULTS = res
    out = np.concatenate([res.results[k]["out"] for k in range(NCORES)], axis=0)
    return np.ascontiguousarray(out.reshape(B, 2 * C, HH, WW))
